# revision 1
# baseline (speedup 1.0000x reference)
"""Trainium2 Bass kernel for nn_AsynBaseStem (sparse 7x7 conv + BN + ReLU +
scatter + 3x3/2 maxpool), 8-core data-parallel over output row bands.

Architecture (per core, fully dense, no indirect DMA):
  - Host prebuilds a [128, 81*646] bf16 operand table T6 per core:
      rows 0..125  : (j,i,ch) j<6 -> fm_pad[r+i, c+j, ch]  (column-shifted planar stripes)
      row  126     : inactive flag (1.0 where pixel has no site, else 0.0)
      row  127     : ones (bias row)
  - Dense conv at every pixel via 2 accumulating matmuls (K=128 main + K=21
    tail read from T6 rows 0..20 at col offset +6). The flag row adds -1e9 to
    inactive pixels (masking), the ones row adds the BN bias.
  - PSUM eviction fuses the column max-pool (DVE even/odd max + ACT third-col
    copy), then a row ring-buffer completes the 3x3/2 max pool.
  - Final ReLU folded into the row pool; one cast-DMA writes [64, p*320] f32;
    the host transposes to [p, q, ch] during unsharding.

kernel(**inputs) takes FULL unsharded inputs, returns [319, 319, 64] f32.
"""
import numpy as np
import ml_dtypes
from contextlib import ExitStack

H = W = 640
CIN, COUT = 3, 64
K, PAD = 7, 3
NCORES = 8
BROWS = 81            # dense rows per core band
WPAD = W + 2 * PAD    # 646
NB = BROWS * WPAD     # T6 free size per core
NBP = NB + 8          # +pad so the tail matmul window (x+6) stays in bounds
PROWS = 40            # pooled rows per core (core 7: 39 valid)
QCOLS = 319
BN_EPS = 1e-5
NEG = -1.0e9


def _build_bass():
    import concourse.bass as bass
    import concourse.mybir as mybir
    import concourse.tile as tile
    from concourse import bacc

    fp32 = mybir.dt.float32
    bf16 = mybir.dt.bfloat16

    nc = bacc.Bacc()
    t6_ext = nc.declare_dram_parameter("t6", [128, NBP], bf16, isOutput=False)
    # packed params: [w | wtail(pad128) | sel126 | sel127 | gam | bet | mu | var]
    par_ext = nc.declare_dram_parameter("par", [128, 8 * COUT], fp32, isOutput=False)
    out_ext = nc.declare_dram_parameter("out", [COUT, PROWS * 320], fp32, isOutput=True)

    with ExitStack() as ctx:
        tc = ctx.enter_context(tile.TileContext(nc))
        cpool = ctx.enter_context(tc.tile_pool(name="const", bufs=1))
        rowp = ctx.enter_context(tc.tile_pool(name="rowbufs", bufs=12))
        ringp = ctx.enter_context(tc.tile_pool(name="ring", bufs=1))
        psp = ctx.enter_context(tc.tile_pool(name="ps", bufs=8, space="PSUM"))

        # ---- weight prep: lhsT A [128, 64] (W'[0:126] + flag row + bias row),
        #      lhsT B [21, 64] (W'[126:147]); W' = W * inv, inv = gamma*rsqrt(var+eps)
        par = cpool.tile([128, 8 * COUT], fp32)
        nc.sync.dma_start(par[:], par_ext[:])
        C = COUT
        wa_f = par[:, 0:C]
        wb_f = par[0:21, C:C + C]
        s126 = par[:, 2 * C:3 * C]
        s127 = par[:, 3 * C:4 * C]
        gam = par[:, 4 * C:5 * C]
        bet = par[:, 5 * C:6 * C]
        mu = par[:, 6 * C:7 * C]
        var = par[:, 7 * C:8 * C]

        # ---- big operand table: chunked load AFTER the small parameter DMAs
        # (HWDGE is FIFO per queue) so weight prep and the first conv rows
        # don't wait for the full 13.4MB stream
        t6 = cpool.tile([128, NBP], bf16)
        bounds = [0, 2] + [2 + 10 * i for i in range(1, 8)] + [BROWS]
        for ck in range(len(bounds) - 1):
            sl = slice(bounds[ck] * WPAD,
                       bounds[ck + 1] * WPAD if ck + 2 < len(bounds) else NBP)
            nc.sync.dma_start(t6[:, sl], t6_ext[:, sl])

        # lhsA = wa*inv + selD*bias' + selN, where selD = sel127 - sel126 and
        # selN = -1e9 at row 126 (host constants); bias' = bet - mu*inv
        inv = cpool.tile([128, COUT], fp32)
        nc.vector.tensor_scalar_add(inv[:], var, BN_EPS)
        nc.scalar.activation(inv[:], inv[:], mybir.ActivationFunctionType.Sqrt)
        nc.vector.reciprocal(inv[:], inv[:])
        nc.vector.tensor_mul(inv[:], inv[:], gam)
        u = cpool.tile([128, COUT], fp32)
        nc.vector.tensor_mul(u[:], mu, inv[:])
        nc.vector.tensor_sub(u[:], bet, u[:])          # u = bias'
        nc.vector.tensor_mul(u[:], u[:], s126)         # u = selD*bias'
        acc = cpool.tile([128, COUT], fp32)
        nc.vector.tensor_mul(acc[:], wa_f, inv[:])
        nc.vector.tensor_add(acc[:], acc[:], s127)     # + selN
        lhsA = cpool.tile([128, COUT], bf16)
        nc.vector.tensor_add(lhsA[:], acc[:], u[:])
        lhsB = cpool.tile([21, COUT], bf16)
        nc.vector.tensor_mul(lhsB[:], wb_f, inv[0:21, :])

        # ---- pooled accumulator [64, PROWS, 320] bf16 and row ring ----
        pooled = ringp.tile([COUT, PROWS * 320], bf16)
        mring = ringp.tile([COUT, 8 * 320], bf16)  # m rows modulo 8

        # Continuous-pixel-space conv: N=512 matmul tiles over x in [0, NB).
        # Row-boundary/pad pixels carry flag=1 -> -1e9, so the pool ignores
        # them. Per-row ev (even cols) and t (pair-max) staging buffers absorb
        # tile fragments; a full-row m then feeds the row pool.
        NT = (NB + 511) // 512
        evrow = {}
        trow = {}

        def finish_row(r):
            mrow = mring[:, (r % 8) * 320:(r % 8) * 320 + 320]
            nc.vector.tensor_tensor(
                out=mrow[:], in0=trow[r][:, 0:320], in1=evrow[r][:, 1:321],
                op=mybir.AluOpType.max)
            del evrow[r], trow[r]
            if r >= 2 and r % 2 == 0:
                p = (r - 2) // 2
                m0 = mring[:, ((r - 2) % 8) * 320:((r - 2) % 8) * 320 + 320]
                m1 = mring[:, ((r - 1) % 8) * 320:((r - 1) % 8) * 320 + 320]
                s01 = rowp.tile([COUT, 320], bf16, tag="s01")
                nc.vector.tensor_tensor(out=s01[:], in0=m0[:], in1=m1[:],
                                        op=mybir.AluOpType.max)
                po = pooled[:, p * 320:(p + 1) * 320]
                nc.vector.scalar_tensor_tensor(
                    out=po[:], in0=s01[:], scalar=0.0, in1=mrow[:],
                    op0=mybir.AluOpType.max, op1=mybir.AluOpType.max)
                # stream pooled rows out in chunks of 10 (cast bf16 -> f32);
                # host does the final [p,q,ch] transpose during unsharding
                if p % 5 == 4:
                    pc = p // 5
                    nc.gpsimd.dma_start(
                        out_ext[:, pc * 1600:(pc + 1) * 1600],
                        pooled[:, pc * 1600:(pc + 1) * 1600])

        for k in range(NT):
            xa = 512 * k
            xb = min(xa + 512, NB)
            wdt = xb - xa
            ps = psp.tile([COUT, 512], fp32, tag="convps")
            nc.tensor.matmul(ps[:, 0:wdt], lhsA[:], t6[0:128, xa:xb],
                             start=True, stop=False)
            nc.tensor.matmul(ps[:, 0:wdt], lhsB[:], t6[0:21, xa + 6:xb + 6],
                             start=False, stop=True)
            for r in range(xa // WPAD, (xb - 1) // WPAD + 1):
                ca = max(xa, r * WPAD) - r * WPAD     # even
                cb = min(xb, r * WPAD + WPAD) - r * WPAD  # even
                if r not in evrow:
                    evrow[r] = rowp.tile([COUT, 324], bf16, tag="evrow", name=f"evrow{r}")
                    trow[r] = rowp.tile([COUT, 324], bf16, tag="trowb", name=f"trowb{r}")
                ne = (cb - ca) // 2
                p0 = r * WPAD + ca - xa               # psum-local offset
                nc.scalar.copy(evrow[r][:, ca // 2:ca // 2 + ne],
                               ps[:, p0:p0 + 2 * ne:2])
                nc.vector.tensor_tensor(
                    out=trow[r][:, ca // 2:ca // 2 + ne],
                    in0=evrow[r][:, ca // 2:ca // 2 + ne],
                    in1=ps[:, p0 + 1:p0 + 2 * ne:2],
                    op=mybir.AluOpType.max)
                if cb == WPAD:
                    finish_row(r)


    nc.finalize()
    return nc


_NC_CACHE = None


def _get_nc():
    global _NC_CACHE
    if _NC_CACHE is None:
        _NC_CACHE = _build_bass()
    return _NC_CACHE


def build_in_maps(update_location, feature_map, weight, gamma, beta,
                  running_mean, running_var):
    fm = np.asarray(feature_map, np.float32)
    loc = np.asarray(update_location).astype(np.int64)
    wt = np.asarray(weight, np.float32)

    fm_pad = np.pad(fm, ((PAD, PAD), (PAD, PAD), (0, 0)))          # [646,646,3]
    # stripes B_T[t=(i,ch), r, c] = fm_pad[r+i, c, ch], r in 0..640 (row 640 pad)
    bt = np.zeros((21, H + 1, WPAD), np.float32)
    for i in range(K):
        for ch in range(CIN):
            bt[i * CIN + ch, 0:H, :] = fm_pad[i:i + H, :, ch]
    bt = bt.astype(ml_dtypes.bfloat16)

    # inactive flag = 1 where no site; indexed by output pixel (r, c) at
    # position c in the 646-pitch row; columns 640..645 stay inactive.
    flag = np.ones((H + 1, WPAD), np.float32)
    flag[loc[:, 0], loc[:, 1]] = 0.0
    flag[:, H:] = 1.0
    flag = flag.astype(ml_dtypes.bfloat16)

    # reordered weights W_re[(j,i,ch), o] = weight[i, j, ch, o]
    w_re = np.ascontiguousarray(
        wt.transpose(1, 0, 2, 3).reshape(147, COUT)).astype(np.float32)

    bcast = lambda v: np.ascontiguousarray(
        np.broadcast_to(np.asarray(v, np.float32)[None, :], (128, COUT)))

    in_maps = []
    for k in range(NCORES):
        r0 = 80 * k
        t6 = np.zeros((128, BROWS, WPAD), ml_dtypes.bfloat16)
        for j in range(6):
            sl = bt[:, r0:r0 + BROWS, :]
            t6[j * 21:(j + 1) * 21, :, :-j or None] = sl[:, :, j:]
        t6[126] = flag[r0:r0 + BROWS]
        t6[127] = np.ones((BROWS, WPAD), ml_dtypes.bfloat16)
        wfull = np.zeros((128, COUT), np.float32)
        wfull[0:126] = w_re[0:126]
        sel126 = np.zeros((128, COUT), np.float32)   # selD: +1 at row 127, -1 at 126
        sel126[127] = 1.0
        sel126[126] = -1.0
        sel127 = np.zeros((128, COUT), np.float32)   # selN: NEG at row 126
        sel127[126] = NEG
        t6p = np.zeros((128, NBP), ml_dtypes.bfloat16)
        t6p[:, :NB] = t6.reshape(128, NB)
        wtail = np.zeros((128, COUT), np.float32)
        wtail[0:21] = w_re[126:147]
        par = np.concatenate([wfull, wtail, sel126, sel127, bcast(gamma),
                              bcast(beta), bcast(running_mean),
                              bcast(running_var)], axis=1)
        in_maps.append({"t6": t6p, "par": np.ascontiguousarray(par)})
    return in_maps


def kernel(update_location, feature_map, weight, gamma, beta, running_mean,
           running_var):
    from concourse.bass_utils import run_bass_kernel_spmd

    in_maps = build_in_maps(update_location, feature_map, weight, gamma, beta,
                            running_mean, running_var)
    nc = _get_nc()
    res = run_bass_kernel_spmd(nc, in_maps, core_ids=list(range(NCORES)))
    # per-core out is [64, PROWS*320] f32 (ch-major); assemble [319, 319, 64]
    parts = []
    for k in range(NCORES):
        o = res.results[k]["out"].reshape(COUT, PROWS, 320)
        parts.append(o.transpose(1, 2, 0)[:, :QCOLS, :])
    out = np.concatenate(parts, axis=0)[:QCOLS]
    return np.ascontiguousarray(out).astype(np.float32)



# revision 4
# speedup vs baseline: 1.8505x; 1.8505x over previous
"""Trainium2 Bass kernel for nn_AsynBaseStem (sparse 7x7 conv + BN + ReLU +
scatter + 3x3/2 maxpool), 8-core data-parallel over output row bands.

v2: each core's 81-row band is split into two 41-row half-bands processed in
parallel on PE column groups (tile_position col 0/64), so every PSUM tile is
[128, 512] = two 512-pixel streams. All eviction/pool engine work runs at 128
partitions (half the per-pixel instruction cost of v1). Matmuls are
phase-batched (4x A-stationary, then 4x tail-stationary per 4-tile batch) to
cut LDWEIGHTS thrash and keep the PE streaming toward full p-state.

Per core:
  - Host builds a [128, 2*(41*646+8)] bf16 operand table: per half-band,
    rows 0..125 = column-shifted planar stripes, row 126 = inactive flag,
    row 127 = ones (bias row). Weights/BN folded into a duplicated [128,128]
    stationary (col group 0 = half 0, col group 1 = half 1).
  - Dense conv at every pixel: K=128 main + K=21 tail accumulating matmuls.
  - Eviction per tile: ACT copies even columns, DVE pair-maxes even/odd
    (both into full-length bf16 rings - row length 646 is even, so pair
    parity never straddles rows). Pool engine does the per-row 3-col max,
    DVE finishes the 3-row max + ReLU, gpsimd cast-DMAs the output.

kernel(**inputs) takes FULL unsharded inputs, returns [319, 319, 64] f32.
"""
import numpy as np
import ml_dtypes
from contextlib import ExitStack

H = W = 640
CIN, COUT = 3, 64
K, PAD = 7, 3
NCORES = 8
HROWS = 41            # dense rows per half-band
WPAD = W + 2 * PAD    # 646
NBH = HROWS * WPAD    # 26486 columns per half-band
NBHP = NBH + 8        # +pad so the tail matmul window (x+6) stays in bounds
HPOOL = 20            # pooled rows per half-band
QCOLS = 319
BN_EPS = 1e-5
NEG = -1.0e9
HCOLS = NBH // 2      # 13243 half-columns (pair stream) per half-band


def _build_bass():
    import concourse.bass as bass
    import concourse.mybir as mybir
    import concourse.tile as tile
    from concourse import bacc

    fp32 = mybir.dt.float32
    bf16 = mybir.dt.bfloat16

    nc = bacc.Bacc()
    t6_ext = nc.declare_dram_parameter("t6", [128, 2 * NBHP], bf16, isOutput=False)
    # packed params: [w | wtail(pad128) | sel126 | sel127 | gam | bet | mu | var]
    par_ext = nc.declare_dram_parameter("par", [128, 8 * COUT], fp32, isOutput=False)
    out_ext = nc.declare_dram_parameter("out", [128, HPOOL * 320], fp32, isOutput=True)

    with ExitStack() as ctx:
        tc = ctx.enter_context(tile.TileContext(nc))
        cpool = ctx.enter_context(tc.tile_pool(name="const", bufs=1))
        rowp = ctx.enter_context(tc.tile_pool(name="rowbufs", bufs=4))
        ringp = ctx.enter_context(tc.tile_pool(name="ring", bufs=1))
        psp = ctx.enter_context(tc.tile_pool(name="ps", bufs=8, space="PSUM"))

        # ---- weight prep: stationary A [128, 128] (two copies of W'[0:126] +
        #      flag row + bias row), stationary B [21, 128] (two copies of
        #      W'[126:147]); W' = W * inv, inv = gamma*rsqrt(var+eps)
        par = cpool.tile([128, 8 * COUT], fp32)
        nc.sync.dma_start(par[:], par_ext[:])
        C = COUT
        wa_f = par[:, 0:C]
        wb_f = par[0:21, C:C + C]
        s126 = par[:, 2 * C:3 * C]
        s127 = par[:, 3 * C:4 * C]
        gam = par[:, 4 * C:5 * C]
        bet = par[:, 5 * C:6 * C]
        mu = par[:, 6 * C:7 * C]
        var = par[:, 7 * C:8 * C]

        # ---- big operand table: chunked load AFTER the small parameter DMAs
        # (HWDGE is FIFO per queue); interleave half-band chunks so the first
        # tile-pairs of BOTH halves unblock early
        t6 = cpool.tile([128, 2 * NBHP], bf16)
        bounds = [0, 2, 7, 12, 22, 32, HROWS]
        for ck in range(len(bounds) - 1):
            a = bounds[ck] * WPAD
            b = bounds[ck + 1] * WPAD if ck + 2 < len(bounds) else NBHP
            nc.sync.dma_start(t6[:, a:b], t6_ext[:, a:b])
            nc.sync.dma_start(t6[:, NBHP + a:NBHP + b], t6_ext[:, NBHP + a:NBHP + b])

        # lhsA = wa*inv + selD*bias' + selN, where selD = sel127 - sel126 and
        # selN = -1e9 at row 126 (host constants); bias' = bet - mu*inv
        inv = cpool.tile([128, COUT], fp32)
        nc.vector.tensor_scalar_add(inv[:], var, BN_EPS)
        nc.scalar.activation(inv[:], inv[:], mybir.ActivationFunctionType.Sqrt)
        nc.vector.reciprocal(inv[:], inv[:])
        nc.vector.tensor_mul(inv[:], inv[:], gam)
        u = cpool.tile([128, COUT], fp32)
        nc.vector.tensor_mul(u[:], mu, inv[:])
        nc.vector.tensor_sub(u[:], bet, u[:])          # u = bias'
        nc.vector.tensor_mul(u[:], u[:], s126)         # u = selD*bias'
        acc = cpool.tile([128, COUT], fp32)
        nc.vector.tensor_mul(acc[:], wa_f, inv[:])
        nc.vector.tensor_add(acc[:], acc[:], s127)     # + selN
        statA = cpool.tile([128, 2 * COUT], bf16)
        nc.vector.tensor_add(statA[:, 0:C], acc[:], u[:])
        nc.vector.tensor_add(statA[:, C:2 * C], acc[:], u[:])
        statB = cpool.tile([21, 2 * COUT], bf16)
        nc.vector.tensor_mul(statB[:, 0:C], wb_f, inv[0:21, :])
        nc.vector.tensor_mul(statB[:, C:2 * C], wb_f, inv[0:21, :])

        # ---- full-length bf16 pair-stream rings + pooled accumulator ----
        ering = ringp.tile([128, HCOLS + 1], bf16)   # even columns
        tring = ringp.tile([128, HCOLS + 1], bf16)   # max(even, odd)
        mring = ringp.tile([128, 8 * 320], bf16)     # per-row 3-col max
        pooled = ringp.tile([128, HPOOL * 320], bf16)

        NT = (NBH + 511) // 512  # 52 tile-pairs
        next_row = [0]

        def finish_rows(xb):
            while (next_row[0] + 1) * WPAD <= xb:
                r = next_row[0]
                next_row[0] += 1
                h = r * (WPAD // 2)
                mrow = mring[:, (r % 8) * 320:(r % 8) * 320 + 320]
                # m[c'] = max(t[c'], e[c'+1]) : 3-col window max for row r
                nc.vector.tensor_tensor(
                    out=mrow[:, 0:QCOLS], in0=tring[:, h:h + QCOLS],
                    in1=ering[:, h + 1:h + 1 + QCOLS], op=mybir.AluOpType.max)
                if r >= 2 and r % 2 == 0:
                    p = (r - 2) // 2
                    m0 = mring[:, ((r - 2) % 8) * 320:((r - 2) % 8) * 320 + 320]
                    m1 = mring[:, ((r - 1) % 8) * 320:((r - 1) % 8) * 320 + 320]
                    s01 = rowp.tile([128, 320], bf16, tag="s01")
                    nc.vector.tensor_tensor(out=s01[:], in0=m0[:], in1=m1[:],
                                            op=mybir.AluOpType.max)
                    po = pooled[:, p * 320:(p + 1) * 320]
                    nc.vector.scalar_tensor_tensor(
                        out=po[:], in0=s01[:], scalar=0.0, in1=mrow[:],
                        op0=mybir.AluOpType.max, op1=mybir.AluOpType.max)
                    # stream pooled rows out in chunks of 5 (cast bf16 -> f32)
                    if p % 5 == 4:
                        pc = p // 5
                        nc.gpsimd.dma_start(
                            out_ext[:, pc * 1600:(pc + 1) * 1600],
                            pooled[:, pc * 1600:(pc + 1) * 1600])

        for b in range(0, NT, 4):
            ks = range(b, min(b + 4, NT))
            pss = {}
            exts = {}
            for k in ks:
                xa = 512 * k
                xb = min(xa + 512, NBH)
                wdt = xb - xa
                ps = psp.tile([128, 512], fp32, tag="convps")
                pss[k] = (ps, xa, xb, wdt)
            # phase A: main matmuls (stationary A held across the batch)
            for k in ks:
                ps, xa, xb, wdt = pss[k]
                nc.tensor.matmul(ps[0:64, 0:wdt], statA[:, 0:C],
                                 t6[0:128, xa:xb], start=True, stop=False)
                nc.tensor.matmul(ps[64:128, 0:wdt], statA[:, C:2 * C],
                                 t6[0:128, NBHP + xa:NBHP + xb],
                                 start=True, stop=False)
            # phase B: tail matmuls
            for k in ks:
                ps, xa, xb, wdt = pss[k]
                nc.tensor.matmul(ps[0:64, 0:wdt], statB[0:21, 0:C],
                                 t6[0:21, xa + 6:xb + 6], start=False, stop=True)
                nc.tensor.matmul(ps[64:128, 0:wdt], statB[0:21, C:2 * C],
                                 t6[0:21, NBHP + xa + 6:NBHP + xb + 6],
                                 start=False, stop=True)
            # eviction: ACT pulls even cols into the ring, DVE maxes them
            # with the PSUM odd cols (only one PSUM operand allowed per op)
            for k in ks:
                ps, xa, xb, wdt = pss[k]
                hw2 = wdt // 2
                ho = xa // 2
                nc.scalar.copy(ering[:, ho:ho + hw2], ps[:, 0:wdt:2])
                nc.vector.tensor_tensor(
                    out=tring[:, ho:ho + hw2], in0=ering[:, ho:ho + hw2],
                    in1=ps[:, 1:wdt:2], op=mybir.AluOpType.max)
                finish_rows(xb)

    nc.finalize()
    return nc


_NC_CACHE = None


def _get_nc():
    global _NC_CACHE
    if _NC_CACHE is None:
        _NC_CACHE = _build_bass()
    return _NC_CACHE


def build_in_maps(update_location, feature_map, weight, gamma, beta,
                  running_mean, running_var):
    fm = np.asarray(feature_map, np.float32)
    loc = np.asarray(update_location).astype(np.int64)
    wt = np.asarray(weight, np.float32)

    fm_pad = np.pad(fm, ((PAD, PAD), (PAD, PAD), (0, 0)))          # [646,646,3]
    # stripes B_T[t=(i,ch), r, c] = fm_pad[r+i, c, ch], r in 0..640 (row 640 pad)
    bt = np.zeros((21, H + 1, WPAD), np.float32)
    for i in range(K):
        for ch in range(CIN):
            bt[i * CIN + ch, 0:H, :] = fm_pad[i:i + H, :, ch]
    bt = bt.astype(ml_dtypes.bfloat16)

    # inactive flag = 1 where no site; indexed by output pixel (r, c) at
    # position c in the 646-pitch row; columns 640..645 stay inactive.
    flag = np.ones((H + 1, WPAD), np.float32)
    flag[loc[:, 0], loc[:, 1]] = 0.0
    flag[:, H:] = 1.0
    flag = flag.astype(ml_dtypes.bfloat16)

    # reordered weights W_re[(j,i,ch), o] = weight[i, j, ch, o]
    w_re = np.ascontiguousarray(
        wt.transpose(1, 0, 2, 3).reshape(147, COUT)).astype(np.float32)

    bcast = lambda v: np.ascontiguousarray(
        np.broadcast_to(np.asarray(v, np.float32)[None, :], (128, COUT)))

    ones_half = np.ones((HROWS, WPAD), ml_dtypes.bfloat16)

    def build_half(r0):
        t6 = np.zeros((128, HROWS, WPAD), ml_dtypes.bfloat16)
        for j in range(6):
            sl = bt[:, r0:r0 + HROWS, :]
            t6[j * 21:(j + 1) * 21, :, :-j or None] = sl[:, :, j:]
        t6[126] = flag[r0:r0 + HROWS]
        t6[127] = ones_half
        t6p = np.zeros((128, NBHP), ml_dtypes.bfloat16)
        t6p[:, :NBH] = t6.reshape(128, NBH)
        return t6p

    wfull = np.zeros((128, COUT), np.float32)
    wfull[0:126] = w_re[0:126]
    sel126 = np.zeros((128, COUT), np.float32)   # selD: +1 at row 127, -1 at 126
    sel126[127] = 1.0
    sel126[126] = -1.0
    sel127 = np.zeros((128, COUT), np.float32)   # selN: NEG at row 126
    sel127[126] = NEG
    wtail = np.zeros((128, COUT), np.float32)
    wtail[0:21] = w_re[126:147]
    par = np.ascontiguousarray(np.concatenate(
        [wfull, wtail, sel126, sel127, bcast(gamma), bcast(beta),
         bcast(running_mean), bcast(running_var)], axis=1))

    in_maps = []
    for k in range(NCORES):
        r0 = 80 * k
        t6p = np.concatenate([build_half(r0), build_half(r0 + 40)], axis=1)
        in_maps.append({"t6": np.ascontiguousarray(t6p), "par": par})
    return in_maps


def kernel(update_location, feature_map, weight, gamma, beta, running_mean,
           running_var):
    from concourse.bass_utils import run_bass_kernel_spmd

    in_maps = build_in_maps(update_location, feature_map, weight, gamma, beta,
                            running_mean, running_var)
    nc = _get_nc()
    res = run_bass_kernel_spmd(nc, in_maps, core_ids=list(range(NCORES)))
    # per-core out is [128, HPOOL*320] f32: partition (h*64+ch), free (p*320+q)
    parts = []
    for k in range(NCORES):
        o = res.results[k]["out"].reshape(2, COUT, HPOOL, 320)
        parts.append(o.transpose(0, 2, 3, 1).reshape(2 * HPOOL, 320, COUT)[:, :QCOLS, :])
    out = np.concatenate(parts, axis=0)[:QCOLS]
    return np.ascontiguousarray(out).astype(np.float32)


# revision 9
# speedup vs baseline: 1.9232x; 1.0393x over previous
"""Trainium2 Bass kernel for nn_AsynBaseStem (sparse 7x7 conv + BN + ReLU +
scatter + 3x3/2 maxpool), 8-core data-parallel over output row bands.

v2: each core's 81-row band is split into two 41-row half-bands processed in
parallel on PE column groups (tile_position col 0/64), so every PSUM tile is
[128, 512] = two 512-pixel streams. All eviction/pool engine work runs at 128
partitions (half the per-pixel instruction cost of v1). Matmuls are
phase-batched (4x A-stationary, then 4x tail-stationary per 4-tile batch) to
cut LDWEIGHTS thrash and keep the PE streaming toward full p-state.

Per core:
  - Host builds a [128, 2*(41*646+8)] bf16 operand table: per half-band,
    rows 0..125 = column-shifted planar stripes, row 126 = inactive flag,
    row 127 = ones (bias row). Weights/BN folded into a duplicated [128,128]
    stationary (col group 0 = half 0, col group 1 = half 1).
  - Dense conv at every pixel: K=128 main + K=21 tail accumulating matmuls.
  - Eviction per tile: ACT copies even columns, DVE pair-maxes even/odd
    (both into full-length bf16 rings - row length 646 is even, so pair
    parity never straddles rows). Pool engine does the per-row 3-col max,
    DVE finishes the 3-row max + ReLU, gpsimd cast-DMAs the output.

kernel(**inputs) takes FULL unsharded inputs, returns [319, 319, 64] f32.
"""
import numpy as np
import ml_dtypes
from contextlib import ExitStack

H = W = 640
CIN, COUT = 3, 64
K, PAD = 7, 3
NCORES = 8
HROWS = 41            # dense rows per half-band
WPAD = W + 2 * PAD    # 646
NBH = HROWS * WPAD    # 26486 columns per half-band
NBHP = NBH + 8        # +pad so the tail matmul window (x+6) stays in bounds
HPOOL = 20            # pooled rows per half-band
QCOLS = 319
BN_EPS = 1e-5
NEG = -1.0e9
HCOLS = NBH // 2      # 13243 half-columns (pair stream) per half-band


def _build_bass():
    import concourse.bass as bass
    import concourse.mybir as mybir
    import concourse.tile as tile
    from concourse import bacc

    fp32 = mybir.dt.float32
    bf16 = mybir.dt.bfloat16

    nc = bacc.Bacc()
    t6_ext = nc.declare_dram_parameter("t6", [128, 2 * NBHP], bf16, isOutput=False)
    # host-folded stationaries (BN scale/bias + flag row baked in, duplicated
    # across both PE column groups)
    sa_ext = nc.declare_dram_parameter("statA", [128, 2 * COUT], bf16, isOutput=False)
    sb_ext = nc.declare_dram_parameter("statB", [21, 2 * COUT], bf16, isOutput=False)
    out_ext = nc.declare_dram_parameter("out", [128, HPOOL * 320], bf16, isOutput=True)

    with ExitStack() as ctx:
        tc = ctx.enter_context(tile.TileContext(nc))
        cpool = ctx.enter_context(tc.tile_pool(name="const", bufs=1))
        rowp = ctx.enter_context(tc.tile_pool(name="rowbufs", bufs=4))
        ringp = ctx.enter_context(tc.tile_pool(name="ring", bufs=1))
        psp = ctx.enter_context(tc.tile_pool(name="ps", bufs=8, space="PSUM"))

        C = COUT
        statA = cpool.tile([128, 2 * COUT], bf16)
        statB = cpool.tile([21, 2 * COUT], bf16)
        nc.sync.dma_start(statA[:], sa_ext[:])
        nc.sync.dma_start(statB[:], sb_ext[:])

        # ---- big operand table: chunked load AFTER the small parameter DMAs
        # (HWDGE is FIFO per queue); interleave half-band chunks so the first
        # tile-pairs of BOTH halves unblock early
        t6 = cpool.tile([128, 2 * NBHP], bf16)
        bounds = [0, 2, 7, 12, 22, 32, HROWS]
        for ck in range(len(bounds) - 1):
            a = bounds[ck] * WPAD
            b = bounds[ck + 1] * WPAD if ck + 2 < len(bounds) else NBHP
            nc.sync.dma_start(t6[:, a:b], t6_ext[:, a:b])
            nc.sync.dma_start(t6[:, NBHP + a:NBHP + b], t6_ext[:, NBHP + a:NBHP + b])

        # ---- full-length bf16 pair-stream rings + pooled accumulator ----
        ering = ringp.tile([128, HCOLS + 1], bf16)   # even columns
        tring = ringp.tile([128, HCOLS + 1], bf16)   # max(even, odd)
        mring = ringp.tile([128, 8 * 320], bf16)     # per-row 3-col max
        pooled = ringp.tile([128, HPOOL * 320], bf16)

        NT = (NBH + 511) // 512  # 52 tile-pairs
        next_row = [0]

        def finish_rows(xb):
            while (next_row[0] + 1) * WPAD <= xb:
                r = next_row[0]
                next_row[0] += 1
                h = r * (WPAD // 2)
                mrow = mring[:, (r % 8) * 320:(r % 8) * 320 + 320]
                # m[c'] = max(t[c'], e[c'+1]) : 3-col window max for row r
                nc.vector.tensor_tensor(
                    out=mrow[:, 0:QCOLS], in0=tring[:, h:h + QCOLS],
                    in1=ering[:, h + 1:h + 1 + QCOLS], op=mybir.AluOpType.max)
                if r >= 2 and r % 2 == 0:
                    p = (r - 2) // 2
                    m0 = mring[:, ((r - 2) % 8) * 320:((r - 2) % 8) * 320 + 320]
                    m1 = mring[:, ((r - 1) % 8) * 320:((r - 1) % 8) * 320 + 320]
                    s01 = rowp.tile([128, 320], bf16, tag="s01")
                    nc.vector.tensor_tensor(out=s01[:], in0=m0[:], in1=m1[:],
                                            op=mybir.AluOpType.max)
                    po = pooled[:, p * 320:(p + 1) * 320]
                    nc.vector.scalar_tensor_tensor(
                        out=po[:], in0=s01[:], scalar=0.0, in1=mrow[:],
                        op0=mybir.AluOpType.max, op1=mybir.AluOpType.max)
                    # stream pooled row-pairs out (bf16; host casts to f32)
                    if p % 2 == 1:
                        pc = p // 2
                        nc.gpsimd.dma_start(
                            out_ext[:, pc * 640:(pc + 1) * 640],
                            pooled[:, pc * 640:(pc + 1) * 640])

        for b in range(0, NT, 4):
            ks = range(b, min(b + 4, NT))
            pss = {}
            exts = {}
            for k in ks:
                xa = 512 * k
                xb = min(xa + 512, NBH)
                wdt = xb - xa
                ps = psp.tile([128, 512], fp32, tag="convps")
                pss[k] = (ps, xa, xb, wdt)
            # phase A: main matmuls (stationary A held across the batch)
            for k in ks:
                ps, xa, xb, wdt = pss[k]
                nc.tensor.matmul(ps[0:64, 0:wdt], statA[:, 0:C],
                                 t6[0:128, xa:xb], start=True, stop=False)
                nc.tensor.matmul(ps[64:128, 0:wdt], statA[:, C:2 * C],
                                 t6[0:128, NBHP + xa:NBHP + xb],
                                 start=True, stop=False)
            # phase B: tail matmuls
            for k in ks:
                ps, xa, xb, wdt = pss[k]
                nc.tensor.matmul(ps[0:64, 0:wdt], statB[0:21, 0:C],
                                 t6[0:21, xa + 6:xb + 6], start=False, stop=True)
                nc.tensor.matmul(ps[64:128, 0:wdt], statB[0:21, C:2 * C],
                                 t6[0:21, NBHP + xa + 6:NBHP + xb + 6],
                                 start=False, stop=True)
            # eviction: ACT pulls even cols into the ring, DVE maxes them
            # with the PSUM odd cols (only one PSUM operand allowed per op)
            for k in ks:
                ps, xa, xb, wdt = pss[k]
                hw2 = wdt // 2
                ho = xa // 2
                nc.scalar.copy(ering[:, ho:ho + hw2], ps[:, 0:wdt:2])
                nc.vector.tensor_tensor(
                    out=tring[:, ho:ho + hw2], in0=ering[:, ho:ho + hw2],
                    in1=ps[:, 1:wdt:2], op=mybir.AluOpType.max)
                finish_rows(xb)

    nc.finalize()
    return nc


_NC_CACHE = None


def _get_nc():
    global _NC_CACHE
    if _NC_CACHE is None:
        _NC_CACHE = _build_bass()
    return _NC_CACHE


def build_in_maps(update_location, feature_map, weight, gamma, beta,
                  running_mean, running_var):
    fm = np.asarray(feature_map, np.float32)
    loc = np.asarray(update_location).astype(np.int64)
    wt = np.asarray(weight, np.float32)

    fm_pad = np.pad(fm, ((PAD, PAD), (PAD, PAD), (0, 0)))          # [646,646,3]
    # stripes B_T[t=(i,ch), r, c] = fm_pad[r+i, c, ch], r in 0..640 (row 640 pad)
    bt = np.zeros((21, H + 1, WPAD), np.float32)
    for i in range(K):
        for ch in range(CIN):
            bt[i * CIN + ch, 0:H, :] = fm_pad[i:i + H, :, ch]
    bt = bt.astype(ml_dtypes.bfloat16)

    # inactive flag = 1 where no site; indexed by output pixel (r, c) at
    # position c in the 646-pitch row; columns 640..645 stay inactive.
    flag = np.ones((H + 1, WPAD), np.float32)
    flag[loc[:, 0], loc[:, 1]] = 0.0
    flag[:, H:] = 1.0
    flag = flag.astype(ml_dtypes.bfloat16)

    # reordered weights W_re[(j,i,ch), o] = weight[i, j, ch, o]
    w_re = np.ascontiguousarray(
        wt.transpose(1, 0, 2, 3).reshape(147, COUT)).astype(np.float32)

    ones_half = np.ones((HROWS, WPAD), ml_dtypes.bfloat16)

    def build_half(r0):
        t6 = np.zeros((128, HROWS, WPAD), ml_dtypes.bfloat16)
        for j in range(6):
            sl = bt[:, r0:r0 + HROWS, :]
            t6[j * 21:(j + 1) * 21, :, :-j or None] = sl[:, :, j:]
        t6[126] = flag[r0:r0 + HROWS]
        t6[127] = ones_half
        t6p = np.zeros((128, NBHP), ml_dtypes.bfloat16)
        t6p[:, :NBH] = t6.reshape(128, NBH)
        return t6p

    # host-folded BN: inv = gamma*rsqrt(var+eps), bias = beta - mean*inv
    inv = (np.asarray(gamma, np.float32) /
           np.sqrt(np.asarray(running_var, np.float32) + BN_EPS))
    bias = np.asarray(beta, np.float32) - np.asarray(running_mean, np.float32) * inv
    sa = np.zeros((128, COUT), np.float32)
    sa[0:126] = w_re[0:126] * inv[None, :]
    sa[126] = NEG          # flag row: inactive pixels -> -1e9
    sa[127] = bias         # ones row: + BN bias
    statA = np.ascontiguousarray(
        np.concatenate([sa, sa], axis=1).astype(ml_dtypes.bfloat16))
    sb = w_re[126:147] * inv[None, :]
    statB = np.ascontiguousarray(
        np.concatenate([sb, sb], axis=1).astype(ml_dtypes.bfloat16))

    in_maps = []
    for k in range(NCORES):
        r0 = 80 * k
        t6p = np.concatenate([build_half(r0), build_half(r0 + 40)], axis=1)
        in_maps.append({"t6": np.ascontiguousarray(t6p),
                        "statA": statA, "statB": statB})
    return in_maps


def kernel(update_location, feature_map, weight, gamma, beta, running_mean,
           running_var):
    from concourse.bass_utils import run_bass_kernel_spmd

    in_maps = build_in_maps(update_location, feature_map, weight, gamma, beta,
                            running_mean, running_var)
    nc = _get_nc()
    res = run_bass_kernel_spmd(nc, in_maps, core_ids=list(range(NCORES)))
    # per-core out is [128, HPOOL*320] bf16: partition (h*64+ch), free (p*320+q)
    parts = []
    for k in range(NCORES):
        o = np.asarray(res.results[k]["out"], np.float32).reshape(
            2, COUT, HPOOL, 320)
        parts.append(o.transpose(0, 2, 3, 1).reshape(2 * HPOOL, 320, COUT)[:, :QCOLS, :])
    out = np.concatenate(parts, axis=0)[:QCOLS]
    return np.ascontiguousarray(out).astype(np.float32)
